# revision 7
# baseline (speedup 1.0000x reference)
"""DRT scorer kernel for Trainium2 (8 NeuronCores, Bass/Tile).

score[b, p] = sum_k alpha[b,k] * <qsub[b,k,:], dsub[p,k,:]>
with qsub/dsub per-slot-L2-normalized outputs of a shared 2-layer MLP
(E=384 -> H=512 -> K*SUB=384) and alpha a softmax over an attention MLP.

Strategy:
  - Fold alpha and query norms into the query side: qmod[b, s] =
    alpha[b, s//64] * qsub_norm[b, s].  Then score = Dnorm @ qmod.T.
  - Shard docs P across 8 cores (data parallel), pad 100000 -> 102400
    (12800/core = 25 tiles x 512 docs).
  - Per doc tile: transpose X to feature-major via PE, MLP in float32r
    (full PE rate at free-dim 512), per-slot norms via a block-diagonal
    ones matmul producing partition-replicated norm^2, fused
    (S + b2) * rinv on DVE, then one scoring matmul per 128-s block.
"""

import sys

sys.path.insert(0, "/opt/trn_rl_repo")

import numpy as np
import concourse.bacc as bacc
import concourse.mybir as mybir
from concourse.tile import TileContext
from concourse.bass_utils import run_bass_kernel_spmd

F32 = mybir.dt.float32
F32R = mybir.dt.float32r
AF = mybir.ActivationFunctionType
ALU = mybir.AluOpType

E, H, KSUB = 384, 512, 384
NSLOT, SUB = 6, 64
AH = 64
B = 64
P_FULL = 100000
N_CORES = 8
TILE = 512
P_PAD = 102400  # 8 * 25 * TILE
P_SHARD = P_PAD // N_CORES  # 12800
NT = P_SHARD // TILE  # 25
EB, HB, SB = E // 128, H // 128, KSUB // 128  # 3, 4, 3
EPS = 1e-12

_CACHE = {}


def _consts():
    eye = np.eye(128, dtype=np.float32)
    # mask[p, j] = 1 iff p//64 == j//64  (block-diagonal 64x64 ones)
    idx = np.arange(128)
    mask = (idx[:, None] // SUB == idx[None, :] // SUB).astype(np.float32)
    # sel[k, sb*128 + j] = 1 iff k == 2*sb + j//64
    sel = np.zeros((NSLOT, KSUB), dtype=np.float32)
    for sb in range(SB):
        for j in range(128):
            sel[2 * sb + j // SUB, sb * 128 + j] = 1.0
    ones6 = np.ones((NSLOT, 128), dtype=np.float32)
    return eye, mask, sel, ones6


def build(nt=NT):
    p_shard = nt * TILE
    nc = bacc.Bacc()

    docs = nc.declare_dram_parameter("docs", [p_shard, E], F32R, isOutput=False)
    q = nc.declare_dram_parameter("q", [B, E], F32R, isOutput=False)
    W1 = nc.declare_dram_parameter("W1", [E, H], F32R, isOutput=False)
    b1 = nc.declare_dram_parameter("b1", [H], F32, isOutput=False)
    W2 = nc.declare_dram_parameter("W2", [H, KSUB], F32R, isOutput=False)
    b2 = nc.declare_dram_parameter("b2", [KSUB], F32, isOutput=False)
    Wa1 = nc.declare_dram_parameter("Wa1", [E, AH], F32R, isOutput=False)
    ba1 = nc.declare_dram_parameter("ba1", [AH], F32, isOutput=False)
    Wa2 = nc.declare_dram_parameter("Wa2", [AH, NSLOT], F32R, isOutput=False)
    ba2 = nc.declare_dram_parameter("ba2", [NSLOT], F32, isOutput=False)
    scores = nc.declare_dram_parameter("scores", [B, p_shard], F32, isOutput=True)

    eye_np, mask_np, sel_np, ones6_np = _consts()
    eye_d = nc.inline_tensor(eye_np, name="eye_d")
    mask_d = nc.inline_tensor(mask_np, name="mask_d")
    sel_d = nc.inline_tensor(sel_np, name="sel_d")
    ones6_d = nc.inline_tensor(ones6_np, name="ones6_d")

    with TileContext(nc) as tc:
        with (
            tc.tile_pool(name="consts", bufs=1) as consts,
            tc.tile_pool(name="qpool", bufs=1) as qpool,
            tc.tile_pool(name="xp", bufs=2) as xp,
            tc.tile_pool(name="xtp", bufs=2) as xtp,
            tc.tile_pool(name="htp", bufs=8) as htp,
            tc.tile_pool(name="sqp", bufs=3) as sqp,
            tc.tile_pool(name="nrmp", bufs=3) as nrmp,
            tc.tile_pool(name="rinp", bufs=3) as rinp,
            tc.tile_pool(name="snp", bufs=6) as snp,
            tc.tile_pool(name="outp", bufs=3) as outp,
            tc.tile_pool(name="pst", bufs=1, space="PSUM") as pst,
            tc.tile_pool(name="psh", bufs=2, space="PSUM") as psh,
            tc.tile_pool(name="pss", bufs=2, space="PSUM") as pss,
            tc.tile_pool(name="psn", bufs=1, space="PSUM") as psn,
            tc.tile_pool(name="psc", bufs=2, space="PSUM") as psc,
        ):
            # ---- constants / weights to SBUF (one-time) ----
            eye = consts.tile([128, 128], F32R)
            nc.gpsimd.dma_start(out=eye, in_=eye_d[:, :])
            mask = consts.tile([128, 128], F32R)
            nc.gpsimd.dma_start(out=mask, in_=mask_d[:, :])
            sel = consts.tile([NSLOT, KSUB], F32R)
            nc.gpsimd.dma_start(out=sel, in_=sel_d[:, :])
            ones6 = consts.tile([NSLOT, 128], F32R)
            nc.gpsimd.dma_start(out=ones6, in_=ones6_d[:, :])

            w1 = consts.tile([128, EB, H], F32R)
            nc.sync.dma_start(out=w1, in_=W1[:, :].rearrange("(eb p) h -> p eb h", p=128))
            w2 = consts.tile([128, HB, KSUB], F32R)
            nc.sync.dma_start(out=w2, in_=W2[:, :].rearrange("(hb p) s -> p hb s", p=128))
            wa1 = consts.tile([128, EB, AH], F32R)
            nc.sync.dma_start(out=wa1, in_=Wa1[:, :].rearrange("(eb p) a -> p eb a", p=128))
            wa2 = consts.tile([AH, NSLOT], F32R)
            nc.sync.dma_start(out=wa2, in_=Wa2[:, :])

            epst = consts.tile([128, 1], F32)
            nc.vector.memset(epst, EPS)

            b1t = consts.tile([128, HB], F32)
            nc.sync.dma_start(out=b1t, in_=b1[:].rearrange("(hb p) -> p hb", p=128))
            b2t = consts.tile([128, SB], F32)
            nc.sync.dma_start(out=b2t, in_=b2[:].rearrange("(sb p) -> p sb", p=128))
            ba1t = consts.tile([AH, 1], F32)
            nc.sync.dma_start(out=ba1t, in_=ba1[:].rearrange("(a one) -> a one", one=1))
            ba2t = consts.tile([NSLOT, 1], F32)
            nc.sync.dma_start(out=ba2t, in_=ba2[:].rearrange("(k one) -> k one", one=1))

            # ---- query phase: build qmodT (128, SB, B) in f32r ----
            qn = qpool.tile([B, E], F32R)
            nc.sync.dma_start(out=qn, in_=q[:, :])

            qt = qpool.tile([128, EB, B], F32R)
            for eb in range(EB):
                tq_ps = pst.tile([128, B], F32R, tag="pst")
                nc.tensor.matmul(
                    tq_ps, qn[:, eb * 128 : (eb + 1) * 128], eye[:B, :B],
                    is_transpose=True,
                )
                nc.vector.tensor_copy(qt[:, eb, :], tq_ps)

            hq = qpool.tile([128, HB, B], F32R)
            for hb in range(HB):
                hq_ps = psh.tile([128, B], F32, tag="psh")
                for eb in range(EB):
                    nc.tensor.matmul(
                        hq_ps,
                        w1[:, eb, hb * 128 : (hb + 1) * 128],
                        qt[:, eb, :],
                        start=(eb == 0),
                        stop=(eb == EB - 1),
                    )
                nc.scalar.activation(
                    out=hq[:, hb, :], in_=hq_ps, func=AF.Relu, bias=b1t[:, hb : hb + 1]
                )

            sq_v = qpool.tile([128, SB, B], F32)  # s + b2 (query)
            rinvq = qpool.tile([128, SB, B], F32)
            for sb in range(SB):
                sq_ps = pss.tile([128, B], F32, tag="pss")
                for hb in range(HB):
                    nc.tensor.matmul(
                        sq_ps,
                        w2[:, hb, sb * 128 : (sb + 1) * 128],
                        hq[:, hb, :],
                        start=(hb == 0),
                        stop=(hb == HB - 1),
                    )
                sqq = qpool.tile([128, B], F32R, tag="sqq")
                nc.scalar.activation(
                    out=sqq, in_=sq_ps, func=AF.Square, bias=b2t[:, sb : sb + 1]
                )
                nc.vector.tensor_scalar_add(sq_v[:, sb, :], sq_ps, b2t[:, sb : sb + 1])
                nq_ps = psn.tile([128, B], F32, tag="psn")
                nc.tensor.matmul(nq_ps, mask, sqq)
                nrmq = qpool.tile([128, B], F32, tag="nrmq")
                nc.scalar.activation(out=nrmq, in_=nq_ps, func=AF.Sqrt, bias=epst[:, 0:1])
                nc.vector.reciprocal(rinvq[:, sb, :], nrmq)

            # alphas
            aq_ps = psh.tile([AH, B], F32, tag="psh")
            for eb in range(EB):
                nc.tensor.matmul(
                    aq_ps, wa1[:, eb, :], qt[:, eb, :],
                    start=(eb == 0), stop=(eb == EB - 1),
                )
            aq = qpool.tile([AH, B], F32R)
            nc.scalar.activation(out=aq, in_=aq_ps, func=AF.Relu, bias=ba1t[:, 0:1])

            lq_ps = pss.tile([NSLOT, B], F32, tag="pss")
            nc.tensor.matmul(lq_ps, wa2, aq)
            eq = qpool.tile([NSLOT, B], F32R)
            nc.scalar.activation(out=eq, in_=lq_ps, func=AF.Exp, bias=ba2t[:, 0:1])

            sum_ps = psn.tile([128, B], F32, tag="psn")
            nc.tensor.matmul(sum_ps, ones6, eq)
            rsum = qpool.tile([128, B], F32)
            nc.vector.reciprocal(rsum, sum_ps)

            qmodT = consts.tile([128, SB, B], F32R)
            for sb in range(SB):
                al_ps = psc.tile([128, B], F32, tag="psc")
                nc.tensor.matmul(al_ps, sel[:, sb * 128 : (sb + 1) * 128], eq)
                alph = qpool.tile([128, B], F32, tag="alph")
                nc.vector.tensor_mul(alph, al_ps, rsum)
                tmpq = qpool.tile([128, B], F32, tag="tmpq")
                nc.vector.tensor_mul(tmpq, sq_v[:, sb, :], rinvq[:, sb, :])
                nc.vector.tensor_mul(qmodT[:, sb, :], tmpq, alph)

            # ---- doc loop ----
            docs_r = docs[:, :].rearrange("(t c p) e -> t p c e", p=128, c=TILE // 128)
            for t in range(nt):
                xn = xp.tile([128, TILE // 128, E], F32R, tag="xn")
                nc.sync.dma_start(out=xn, in_=docs_r[t])

                xt = xtp.tile([128, EB, TILE], F32R, tag="xt")
                for eb in range(EB):
                    tx_ps = pst.tile([128, TILE], F32R, tag="pst")
                    for c in range(TILE // 128):
                        nc.tensor.matmul(
                            tx_ps[:, c * 128 : (c + 1) * 128],
                            xn[:, c, eb * 128 : (eb + 1) * 128],
                            eye,
                            is_transpose=True,
                        )
                    nc.vector.tensor_copy(xt[:, eb, :], tx_ps)

                hts = []
                for hb in range(HB):
                    h_ps = psh.tile([128, TILE], F32, tag="psh")
                    for eb in range(EB):
                        nc.tensor.matmul(
                            h_ps,
                            w1[:, eb, hb * 128 : (hb + 1) * 128],
                            xt[:, eb, :],
                            start=(eb == 0),
                            stop=(eb == EB - 1),
                        )
                    ht = htp.tile([128, TILE], F32R, tag="ht")
                    nc.scalar.activation(
                        out=ht, in_=h_ps, func=AF.Relu, bias=b1t[:, hb : hb + 1]
                    )
                    hts.append(ht)

                sc_ps = psc.tile([B, TILE], F32, tag="psc")
                for sb in range(SB):
                    s_ps = pss.tile([128, TILE], F32, tag="pss")
                    for hb in range(HB):
                        nc.tensor.matmul(
                            s_ps,
                            w2[:, hb, sb * 128 : (sb + 1) * 128],
                            hts[hb],
                            start=(hb == 0),
                            stop=(hb == HB - 1),
                        )
                    sqd = sqp.tile([128, TILE], F32R, tag="sqd")
                    nc.scalar.activation(
                        out=sqd, in_=s_ps, func=AF.Square, bias=b2t[:, sb : sb + 1]
                    )
                    n_ps = psn.tile([128, TILE], F32, tag="psn")
                    nc.tensor.matmul(n_ps, mask, sqd)
                    nrm = nrmp.tile([128, TILE], F32, tag="nrm")
                    nc.scalar.activation(out=nrm, in_=n_ps, func=AF.Sqrt, bias=epst[:, 0:1])
                    rin = rinp.tile([128, TILE], F32, tag="rin")
                    nc.vector.reciprocal(rin, nrm)
                    sn = snp.tile([128, TILE], F32R, tag="sn")
                    nc.vector.scalar_tensor_tensor(
                        out=sn, in0=s_ps, scalar=b2t[:, sb : sb + 1], in1=rin,
                        op0=ALU.add, op1=ALU.mult,
                    )
                    nc.tensor.matmul(
                        sc_ps, qmodT[:, sb, :], sn,
                        start=(sb == 0), stop=(sb == SB - 1),
                    )

                ot = outp.tile([B, TILE], F32, tag="ot")
                nc.scalar.copy(ot, sc_ps)
                nc.sync.dma_start(
                    out=scores[:, t * TILE : (t + 1) * TILE], in_=ot
                )

    nc.compile()
    return nc


def kernel(
    query_emb, doc_emb, W1, b1, W2, b2, Wa1, ba1, Wa2, ba2
):
    if "nc" not in _CACHE:
        _CACHE["nc"] = build()
    nc = _CACHE["nc"]

    docs = np.ascontiguousarray(doc_emb.reshape(P_FULL, E), dtype=np.float32)
    docs_pad = np.zeros((P_PAD, E), dtype=np.float32)
    docs_pad[:P_FULL] = docs

    common = {
        "q": np.ascontiguousarray(query_emb.reshape(B, E), dtype=np.float32),
        "W1": np.ascontiguousarray(W1, dtype=np.float32),
        "b1": np.ascontiguousarray(b1, dtype=np.float32),
        "W2": np.ascontiguousarray(W2, dtype=np.float32),
        "b2": np.ascontiguousarray(b2, dtype=np.float32),
        "Wa1": np.ascontiguousarray(Wa1, dtype=np.float32),
        "ba1": np.ascontiguousarray(ba1, dtype=np.float32),
        "Wa2": np.ascontiguousarray(Wa2, dtype=np.float32),
        "ba2": np.ascontiguousarray(ba2, dtype=np.float32),
    }
    in_maps = []
    for i in range(N_CORES):
        m = dict(common)
        m["docs"] = np.ascontiguousarray(
            docs_pad[i * P_SHARD : (i + 1) * P_SHARD]
        )
        in_maps.append(m)

    trace = _CACHE.get("trace", False)
    res = run_bass_kernel_spmd(
        nc, in_maps, core_ids=list(range(N_CORES)), trace=trace
    )
    _CACHE["last_result"] = res

    out = np.concatenate([res.results[i]["scores"] for i in range(N_CORES)], axis=1)
    return out[:, :P_FULL]


# revision 8
# speedup vs baseline: 1.2193x; 1.2193x over previous
"""DRT scorer kernel for Trainium2 (8 NeuronCores, Bass/Tile).

score[b, p] = sum_k alpha[b,k] * <qsub[b,k,:], dsub[p,k,:]>
with qsub/dsub per-slot-L2-normalized outputs of a shared 2-layer MLP
(E=384 -> H=512 -> K*SUB=384) and alpha a softmax over an attention MLP.

Strategy:
  - Fold alpha and query norms into the query side: qmod[b, s] =
    alpha[b, s//64] * qsub_norm[b, s].  Then score = Dnorm @ qmod.T.
  - Shard docs P across 8 cores (data parallel), pad 100000 -> 102400
    (12800/core = 25 tiles x 512 docs).
  - bf16 matmul operands (1 cycle/row + fast weight load keeps the PE
    HAM-warm), fp32 PSUM accumulation.
  - Per-slot doc norms via a block-diagonal ones matmul producing
    partition-replicated norm^2; 1/sqrt via ACT exp(-0.5*ln(x)) (the DVE
    reciprocal is an iterative-divide op, ~3.2us per 512-col tile).
  - Elementwise spread across DVE (relu, +b2), GPSIMD (square, scale),
    ACT (ln, exp, output copy).
"""

import sys

sys.path.insert(0, "/opt/trn_rl_repo")

import numpy as np
import concourse.bacc as bacc
import concourse.mybir as mybir
from concourse.tile import TileContext
from concourse.bass_utils import run_bass_kernel_spmd

F32 = mybir.dt.float32
BF16 = mybir.dt.bfloat16
AF = mybir.ActivationFunctionType
ALU = mybir.AluOpType

E, H, KSUB = 384, 512, 384
NSLOT, SUB = 6, 64
AH = 64
B = 64
P_FULL = 100000
N_CORES = 8
TILE = 512
P_PAD = 102400  # 8 * 25 * TILE
P_SHARD = P_PAD // N_CORES  # 12800
NT = P_SHARD // TILE  # 25
EB, HB, SB = E // 128, H // 128, KSUB // 128  # 3, 4, 3
EPS = 1e-12

TRANSPOSE_MODE = "pe"  # "pe" or "dma"

_CACHE = {}


def _consts():
    eye = np.eye(128, dtype=np.float32)
    # mask[p, j] = 1 iff p//64 == j//64  (block-diagonal 64x64 ones)
    idx = np.arange(128)
    mask = (idx[:, None] // SUB == idx[None, :] // SUB).astype(np.float32)
    # sel[k, sb*128 + j] = 1 iff k == 2*sb + j//64
    sel = np.zeros((NSLOT, KSUB), dtype=np.float32)
    for sb in range(SB):
        for j in range(128):
            sel[2 * sb + j // SUB, sb * 128 + j] = 1.0
    ones6 = np.ones((NSLOT, 128), dtype=np.float32)
    return eye, mask, sel, ones6


def build(nt=NT, transpose_mode=None):
    if transpose_mode is None:
        transpose_mode = TRANSPOSE_MODE
    p_shard = nt * TILE
    nc = bacc.Bacc()

    docs = nc.declare_dram_parameter("docs", [p_shard, E], F32, isOutput=False)
    q = nc.declare_dram_parameter("q", [B, E], F32, isOutput=False)
    W1 = nc.declare_dram_parameter("W1", [E, H], F32, isOutput=False)
    b1 = nc.declare_dram_parameter("b1", [H], F32, isOutput=False)
    W2 = nc.declare_dram_parameter("W2", [H, KSUB], F32, isOutput=False)
    b2 = nc.declare_dram_parameter("b2", [KSUB], F32, isOutput=False)
    Wa1 = nc.declare_dram_parameter("Wa1", [E, AH], F32, isOutput=False)
    ba1 = nc.declare_dram_parameter("ba1", [AH], F32, isOutput=False)
    Wa2 = nc.declare_dram_parameter("Wa2", [AH, NSLOT], F32, isOutput=False)
    ba2 = nc.declare_dram_parameter("ba2", [NSLOT], F32, isOutput=False)
    scores = nc.declare_dram_parameter("scores", [B, p_shard], F32, isOutput=True)

    eye_np, mask_np, sel_np, ones6_np = _consts()
    eye_d = nc.inline_tensor(eye_np, name="eye_d")
    mask_d = nc.inline_tensor(mask_np, name="mask_d")
    sel_d = nc.inline_tensor(sel_np, name="sel_d")
    ones6_d = nc.inline_tensor(ones6_np, name="ones6_d")

    with TileContext(nc) as tc:
        with (
            tc.tile_pool(name="consts", bufs=1) as consts,
            tc.tile_pool(name="qpool", bufs=1) as qpool,
            tc.tile_pool(name="xp", bufs=3) as xp,
            tc.tile_pool(name="xtp", bufs=3) as xtp,
            tc.tile_pool(name="htp", bufs=8) as htp,
            tc.tile_pool(name="sn0p", bufs=6) as sn0p,
            tc.tile_pool(name="sqp", bufs=3) as sqp,
            tc.tile_pool(name="lnp", bufs=3) as lnp,
            tc.tile_pool(name="rip", bufs=3) as rip,
            tc.tile_pool(name="snp", bufs=6) as snp,
            tc.tile_pool(name="outp", bufs=3) as outp,
            tc.tile_pool(name="pst", bufs=1, space="PSUM") as pst,
            tc.tile_pool(name="psh", bufs=2, space="PSUM") as psh,
            tc.tile_pool(name="pss", bufs=2, space="PSUM") as pss,
            tc.tile_pool(name="psn", bufs=2, space="PSUM") as psn,
            tc.tile_pool(name="psc", bufs=1, space="PSUM") as psc,
        ):
            # ---- constants / weights to SBUF (one-time, SWDGE casts) ----
            eye = consts.tile([128, 128], BF16)
            nc.gpsimd.dma_start(out=eye, in_=eye_d[:, :])
            mask = consts.tile([128, 128], BF16)
            nc.gpsimd.dma_start(out=mask, in_=mask_d[:, :])
            sel = consts.tile([NSLOT, KSUB], BF16)
            nc.gpsimd.dma_start(out=sel, in_=sel_d[:, :])
            ones6 = consts.tile([NSLOT, 128], BF16)
            nc.gpsimd.dma_start(out=ones6, in_=ones6_d[:, :])

            w1 = consts.tile([128, EB, H], BF16)
            nc.gpsimd.dma_start(out=w1, in_=W1[:, :].rearrange("(eb p) h -> p eb h", p=128))
            w2 = consts.tile([128, HB, KSUB], BF16)
            nc.gpsimd.dma_start(out=w2, in_=W2[:, :].rearrange("(hb p) s -> p hb s", p=128))
            wa1 = consts.tile([128, EB, AH], BF16)
            nc.gpsimd.dma_start(out=wa1, in_=Wa1[:, :].rearrange("(eb p) a -> p eb a", p=128))
            wa2 = consts.tile([AH, NSLOT], BF16)
            nc.gpsimd.dma_start(out=wa2, in_=Wa2[:, :])

            epst = consts.tile([128, 1], F32)
            nc.vector.memset(epst, EPS)

            b1t = consts.tile([128, HB], F32)
            nc.sync.dma_start(out=b1t, in_=b1[:].rearrange("(hb p) -> p hb", p=128))
            b2t = consts.tile([128, SB], F32)
            nc.sync.dma_start(out=b2t, in_=b2[:].rearrange("(sb p) -> p sb", p=128))
            ba1t = consts.tile([AH, 1], F32)
            nc.sync.dma_start(out=ba1t, in_=ba1[:].rearrange("(a one) -> a one", one=1))
            ba2t = consts.tile([NSLOT, 1], F32)
            nc.sync.dma_start(out=ba2t, in_=ba2[:].rearrange("(k one) -> k one", one=1))

            # ---- query phase: build qmodT (128, SB, B) in bf16 ----
            qn = qpool.tile([B, E], BF16)
            nc.gpsimd.dma_start(out=qn, in_=q[:, :])

            qt = qpool.tile([128, EB, B], BF16)
            for eb in range(EB):
                tq_ps = pst.tile([128, B], BF16, tag="pst")
                nc.tensor.matmul(
                    tq_ps, qn[:, eb * 128 : (eb + 1) * 128], eye[:B, :B],
                    is_transpose=True,
                )
                nc.vector.tensor_copy(qt[:, eb, :], tq_ps)

            hq = qpool.tile([128, HB, B], BF16)
            for hb in range(HB):
                hq_ps = psh.tile([128, B], F32, tag="psh")
                for eb in range(EB):
                    nc.tensor.matmul(
                        hq_ps,
                        w1[:, eb, hb * 128 : (hb + 1) * 128],
                        qt[:, eb, :],
                        start=(eb == 0),
                        stop=(eb == EB - 1),
                    )
                nc.scalar.activation(
                    out=hq[:, hb, :], in_=hq_ps, func=AF.Relu, bias=b1t[:, hb : hb + 1]
                )

            sq_v = qpool.tile([128, SB, B], F32)  # s + b2 (query)
            rinvq = qpool.tile([128, SB, B], F32)
            for sb in range(SB):
                sq_ps = pss.tile([128, B], F32, tag="pss")
                for hb in range(HB):
                    nc.tensor.matmul(
                        sq_ps,
                        w2[:, hb, sb * 128 : (sb + 1) * 128],
                        hq[:, hb, :],
                        start=(hb == 0),
                        stop=(hb == HB - 1),
                    )
                sqq = qpool.tile([128, B], BF16, tag="sqq")
                nc.scalar.activation(
                    out=sqq, in_=sq_ps, func=AF.Square, bias=b2t[:, sb : sb + 1]
                )
                nc.vector.tensor_scalar_add(sq_v[:, sb, :], sq_ps, b2t[:, sb : sb + 1])
                nq_ps = psn.tile([128, B], F32, tag="psn")
                nc.tensor.matmul(nq_ps, mask, sqq)
                nrmq = qpool.tile([128, B], F32, tag="nrmq")
                nc.scalar.activation(out=nrmq, in_=nq_ps, func=AF.Sqrt, bias=epst[:, 0:1])
                nc.vector.reciprocal(rinvq[:, sb, :], nrmq)

            # alphas
            aq_ps = psh.tile([AH, B], F32, tag="psh")
            for eb in range(EB):
                nc.tensor.matmul(
                    aq_ps, wa1[:, eb, :], qt[:, eb, :],
                    start=(eb == 0), stop=(eb == EB - 1),
                )
            aq = qpool.tile([AH, B], BF16)
            nc.scalar.activation(out=aq, in_=aq_ps, func=AF.Relu, bias=ba1t[:, 0:1])

            lq_ps = pss.tile([NSLOT, B], F32, tag="pss")
            nc.tensor.matmul(lq_ps, wa2, aq)
            eq = qpool.tile([NSLOT, B], BF16)
            nc.scalar.activation(out=eq, in_=lq_ps, func=AF.Exp, bias=ba2t[:, 0:1])

            sum_ps = psn.tile([128, B], F32, tag="psn")
            nc.tensor.matmul(sum_ps, ones6, eq)
            rsum = qpool.tile([128, B], F32)
            nc.vector.reciprocal(rsum, sum_ps)

            qmodT = consts.tile([128, SB, B], BF16)
            for sb in range(SB):
                al_ps = psc.tile([128, B], F32, tag="psc")
                nc.tensor.matmul(al_ps, sel[:, sb * 128 : (sb + 1) * 128], eq)
                alph = qpool.tile([128, B], F32, tag="alph")
                nc.vector.tensor_mul(alph, al_ps, rsum)
                tmpq = qpool.tile([128, B], F32, tag="tmpq")
                nc.vector.tensor_mul(tmpq, sq_v[:, sb, :], rinvq[:, sb, :])
                nc.vector.tensor_mul(qmodT[:, sb, :], tmpq, alph)

            # ---- doc loop ----
            docs_r = docs[:, :].rearrange("(t c p) e -> t p c e", p=128, c=TILE // 128)
            for t in range(nt):
                xn = xp.tile([128, TILE // 128, E], BF16, tag="xn")
                nc.gpsimd.dma_start(out=xn, in_=docs_r[t])

                xt = xtp.tile([128, EB, TILE], BF16, tag="xt")
                for eb in range(EB):
                    if transpose_mode == "pe":
                        tx_ps = pst.tile([128, TILE], BF16, tag="pst")
                        for c in range(TILE // 128):
                            nc.tensor.matmul(
                                tx_ps[:, c * 128 : (c + 1) * 128],
                                xn[:, c, eb * 128 : (eb + 1) * 128],
                                eye,
                                is_transpose=True,
                            )
                        nc.vector.tensor_copy(xt[:, eb, :], tx_ps)
                    else:
                        for c in range(TILE // 128):
                            nc.sync.dma_start_transpose(
                                out=xt[:, eb, c * 128 : (c + 1) * 128],
                                in_=xn[:, c, eb * 128 : (eb + 1) * 128],
                            )

                hts = []
                for hb in range(HB):
                    h_ps = psh.tile([128, TILE], F32, tag="psh")
                    for eb in range(EB):
                        nc.tensor.matmul(
                            h_ps,
                            w1[:, eb, hb * 128 : (hb + 1) * 128],
                            xt[:, eb, :],
                            start=(eb == 0),
                            stop=(eb == EB - 1),
                        )
                    ht = htp.tile([128, TILE], BF16, tag="ht")
                    nc.vector.tensor_scalar(
                        out=ht, in0=h_ps, scalar1=b1t[:, hb : hb + 1], scalar2=0.0,
                        op0=ALU.add, op1=ALU.max,
                    )
                    hts.append(ht)

                sc_ps = psc.tile([B, TILE], F32, tag="psc")
                for sb in range(SB):
                    s_ps = pss.tile([128, TILE], F32, tag="pss")
                    for hb in range(HB):
                        nc.tensor.matmul(
                            s_ps,
                            w2[:, hb, sb * 128 : (sb + 1) * 128],
                            hts[hb],
                            start=(hb == 0),
                            stop=(hb == HB - 1),
                        )
                    sn0 = sn0p.tile([128, TILE], BF16, tag="sn0")
                    nc.vector.tensor_scalar_add(sn0, s_ps, b2t[:, sb : sb + 1])
                    sq = sqp.tile([128, TILE], BF16, tag="sq")
                    nc.gpsimd.tensor_mul(sq, sn0, sn0)
                    n_ps = psn.tile([128, TILE], F32, tag="psn")
                    nc.tensor.matmul(n_ps, mask, sq)
                    lnn = lnp.tile([128, TILE], F32, tag="lnn")
                    nc.scalar.activation(out=lnn, in_=n_ps, func=AF.Ln, bias=epst[:, 0:1])
                    rin = rip.tile([128, TILE], BF16, tag="rin")
                    nc.scalar.activation(out=rin, in_=lnn, func=AF.Exp, scale=-0.5)
                    sn = snp.tile([128, TILE], BF16, tag="sn")
                    nc.gpsimd.tensor_mul(sn, sn0, rin)
                    nc.tensor.matmul(
                        sc_ps, qmodT[:, sb, :], sn,
                        start=(sb == 0), stop=(sb == SB - 1),
                    )

                ot = outp.tile([B, TILE], F32, tag="ot")
                nc.scalar.copy(ot, sc_ps)
                nc.sync.dma_start(
                    out=scores[:, t * TILE : (t + 1) * TILE], in_=ot
                )

    nc.compile()
    return nc


def kernel(
    query_emb, doc_emb, W1, b1, W2, b2, Wa1, ba1, Wa2, ba2
):
    if "nc" not in _CACHE:
        _CACHE["nc"] = build()
    nc = _CACHE["nc"]

    docs = np.ascontiguousarray(doc_emb.reshape(P_FULL, E), dtype=np.float32)
    docs_pad = np.zeros((P_PAD, E), dtype=np.float32)
    docs_pad[:P_FULL] = docs

    common = {
        "q": np.ascontiguousarray(query_emb.reshape(B, E), dtype=np.float32),
        "W1": np.ascontiguousarray(W1, dtype=np.float32),
        "b1": np.ascontiguousarray(b1, dtype=np.float32),
        "W2": np.ascontiguousarray(W2, dtype=np.float32),
        "b2": np.ascontiguousarray(b2, dtype=np.float32),
        "Wa1": np.ascontiguousarray(Wa1, dtype=np.float32),
        "ba1": np.ascontiguousarray(ba1, dtype=np.float32),
        "Wa2": np.ascontiguousarray(Wa2, dtype=np.float32),
        "ba2": np.ascontiguousarray(ba2, dtype=np.float32),
    }
    in_maps = []
    for i in range(N_CORES):
        m = dict(common)
        m["docs"] = np.ascontiguousarray(
            docs_pad[i * P_SHARD : (i + 1) * P_SHARD]
        )
        in_maps.append(m)

    trace = _CACHE.get("trace", False)
    res = run_bass_kernel_spmd(
        nc, in_maps, core_ids=list(range(N_CORES)), trace=trace
    )
    _CACHE["last_result"] = res

    out = np.concatenate([res.results[i]["scores"] for i in range(N_CORES)], axis=1)
    return out[:, :P_FULL]
